# revision 49
# baseline (speedup 1.0000x reference)
"""Trainium2 Bass kernel for nn_DSVDD (retrieval_knn).

Math (per batch b):
  phi = W @ p_b + bias            [DIM, HW]    (1x1 conv)
  sqdist[i,j] = ||phi_i||^2 + ||C_j||^2 - 2 phi_i . C_j
  top-3 smallest distances d0<=d1<=d2  ->  w0 = 1/(1+exp(d0-d1)+exp(d0-d2))
  score[i] = w0 * d0

Device strategy (8 cores, data-parallel over (batch, HW-half)):
  All heavy matmuls run in fp8e4 with perf_mode=DoubleRow (2 contraction
  rows/cycle, ~1.7x over fp32r at FD>=256).  Host prescales W by 64 and C
  by 128 so fp8 operands sit in the normal range (sigma ~1-3); the scale
  is divided back out on the ACT path (phi = psum/64) and in the final
  sqrt (d = sqrt(f - psum/64)).  Y[i,j] = 64*(2 phi_i . C_j - c_j); the
  -c_j correction is materialized once per j-slice via a ones-matmul on
  the replicated -c/2 block and applied by a DVE add.  top-3 smallest
  sqdist == top-3 largest Y (f_i = ||phi_i||^2 common per row).  DVE max8
  finds the top-8 largest Y per row; streamed merge over j-slices.  f_i
  via fp32 ones-matmuls over Square(psum/64) so f keeps full accuracy.
  Tail (sqrt, softmin) on ACT/DVE.  rel-err budget 2e-2 >> fp8 noise.
"""
import sys

sys.path.insert(0, "/opt/trn_rl_repo")

import numpy as np
import ml_dtypes

B, DIM, H, W_ = 4, 1792, 56, 56
HW = H * W_            # 3136
P = 3136               # prototypes
NCORES = 8
HALF = HW // 2         # 1568 positions per core
KC = DIM // 128        # 14 contraction chunks
NPAIR = KC // 2        # 7 DoubleRow pairs
KCC = KC + 1           # 15 chunks in cb (incl. replicated -c/2 block)
IB = 392               # conv i-block (moving cols)
IBPAD = 400            # p tile inner pad (DoubleRow needs 16B-mult stride)
NIB = HALF // IB       # 4
JSLICES = [480, 480, 480, 480, 480, 512, 224]   # G-phase j-slices (sum 3136)
NJS = len(JSLICES)
NIT = 13               # i-tiles: 12 full + 1 ragged(32)
LAST_W = HALF - 12 * 128   # 32
PHIPAD = NIT * 128     # 1664 (phi padded so the ragged i-tile is uniform)
WSCALE = 64.0          # host prescale on W (and 2C -> 128C)

_cache = {}


def _build_program():
    import concourse.tile as tile
    from concourse import bacc, mybir

    F32 = mybir.dt.float32
    F32R = mybir.dt.float32r
    BF16 = mybir.dt.bfloat16
    F8 = mybir.dt.float8e4
    AF = mybir.ActivationFunctionType
    ALU = mybir.AluOpType
    AX = mybir.AxisListType
    DR = mybir.MatmulPerfMode.DoubleRow

    nc = bacc.Bacc("TRN2", target_bir_lowering=False, debug=False)

    pT_d = nc.dram_tensor("pT", [NIB, 128, KC, IBPAD], F8,
                          kind="ExternalInput")
    wt_d = nc.dram_tensor("wt", [KC, 128, KC, 128], F8, kind="ExternalInput")
    cb_d = nc.dram_tensor("cb", [KCC * 128, P], F8, kind="ExternalInput")
    oner_d = nc.dram_tensor("oner", [1, 128], F32R, kind="ExternalInput")
    ones2_d = nc.dram_tensor("ones2", [128, 2, 64], F8, kind="ExternalInput")
    score_d = nc.dram_tensor("score", [128, NIT], F32, kind="ExternalOutput")

    with tile.TileContext(nc) as tc:
        with (
            tc.tile_pool(name="persist", bufs=1) as persist,
            tc.tile_pool(name="cbp0", bufs=1) as cbp0,
        ):
            phi = persist.tile([128, KC, PHIPAD], F8)
            oner = persist.tile([1, 128], F32R)
            ones2 = persist.tile([128, 2, 64], F8)
            f_row = persist.tile([1, HALF], F32)
            f_col = persist.tile([128, NIT], F32)
            runA = persist.tile([128, NIT, 8 * NJS], F32)
            topA = persist.tile([128, NIT, 8], F32)
            score_col = persist.tile([128, NIT], F32)

            # ------------- conv phase: phi = W @ p + b, f = ||phi||^2 -------
            with (
                tc.tile_pool(name="pp", bufs=6) as pp,
                tc.tile_pool(name="wtp", bufs=3) as wtp,
                tc.tile_pool(name="sqp", bufs=4) as sqp,
                tc.tile_pool(name="cps", bufs=4, space="PSUM") as cps,
                tc.tile_pool(name="fps", bufs=1, space="PSUM") as fps,
            ):
                f_ps = [fps.tile([1, IB], F32, name=f"fp{ib}", tag=f"f{ib}")
                        for ib in range(NIB)]

                def load_wt(dcg):
                    t = wtp.tile([128, KC, 128], F8, name="wt_t")
                    nc.sync.dma_start(t[:], wt_d[dcg])
                    return t

                # host pre-transposes p to ib-major [4, 128, 14, 400], so
                # each i-block loads as ONE DMA of 128 fully-contiguous
                # 5.6KB runs (~2us) instead of ~1k small packets (~6.5us)
                p_tiles = {}

                def load_ptile(ib):
                    # two partition-half DMAs land on different queues and
                    # transfer in parallel (~2x: one queue moves ~130 GB/s)
                    t = pp.tile([128, KC, IBPAD], F8, name=f"pq{ib}",
                                tag="pq")
                    for h in (0, 64):
                        nc.sync.dma_start(t[h:h + 64, :, :],
                                          pT_d[ib, h:h + 64])
                    p_tiles[ib] = t

                # startup-critical loads first: wt chunk 0, then p halves
                dcg_seq = list(range(KC)) + list(reversed(range(KC)))  # snake
                wt_tiles = {0: load_wt(dcg_seq[0])}
                wt_issued = 1

                def wt_prefetch(upto):
                    nonlocal wt_issued
                    while wt_issued < min(upto, 2 * KC):
                        if dcg_seq[wt_issued] == dcg_seq[wt_issued - 1]:
                            # snake turn: same chunk again, reuse the tile
                            wt_tiles[wt_issued] = wt_tiles[wt_issued - 1]
                        else:
                            wt_tiles[wt_issued] = load_wt(dcg_seq[wt_issued])
                        wt_issued += 1

                # startup DMA order: first conv group needs p(ib0) + wt0
                # (wt0 already issued first above)
                load_ptile(0)
                load_ptile(1)
                wt_prefetch(2)
                load_ptile(2)
                load_ptile(3)
                wt_prefetch(3)
                nc.sync.dma_start(oner[:], oner_d[:])
                nc.sync.dma_start(ones2[:], ones2_d[:])

                # PE warmup: dummy matmuls keep HAM's activity monitor hot
                # while the first real DMAs land, so conv starts at full clock.
                warm = pp.tile([128, 512], F32R, name="warm", tag="warm", bufs=1)
                nc.vector.memset(warm[:].bitcast(F32), 1.0)
                wps = cps.tile([128, 512], F32, name="wps", tag="acc")
                for _ in range(10):
                    nc.tensor.matmul(wps[:], warm[:, 0:128], warm[:],
                                     start=True, stop=True)
                # zero the phi pad columns so the uniform last i-tile reads 0s
                nc.vector.memset(phi[:, :, HALF:PHIPAD].bitcast(F32), 0.0)

                cb0_t = None
                pending_f = []
                _sq_cur = {}
                for sub in range(2):
                    for dcg_i in range(KC):
                        pos = sub * KC + dcg_i
                        dcg = dcg_seq[pos]
                        wt_t = wt_tiles.pop(pos)
                        wt_prefetch(pos + 3)
                        for k, ib in enumerate((2 * sub, 2 * sub + 1)):
                            if k == 1 and pending_f:
                                # deferred f matmuls: deps long satisfied
                                for args, kw in pending_f:
                                    nc.tensor.matmul(*args, **kw)
                                pending_f = []
                            acc = cps.tile([128, IB], F32)
                            for pc in range(NPAIR):
                                nc.tensor.matmul(
                                    acc[:],
                                    wt_t[:, 2 * pc:2 * pc + 2, :],
                                    p_tiles[ib][:, 2 * pc:2 * pc + 2, 0:IB],
                                    start=(pc == 0),
                                    stop=(pc == NPAIR - 1),
                                    perf_mode=DR,
                                )
                            isl = slice(ib * IB, (ib + 1) * IB)
                            # phi = psum/64 (bias folded into C on host)
                            nc.scalar.activation(
                                phi[:, dcg, isl], acc[:], AF.Identity,
                                scale=1.0 / WSCALE,
                            )
                            # phi2 = (psum/64)^2 in fp8 (values 0..~30),
                            # paired along dcg for a DoubleRow f-matmul
                            if dcg_i % 2 == 0:
                                sq = sqp.tile([128, 2, IBPAD], F8)
                                _sq_cur[ib] = sq
                            else:
                                sq = _sq_cur[ib]
                            nc.scalar.activation(
                                sq[:, dcg_i % 2, 0:IB], acc[:], AF.Square,
                                scale=1.0 / WSCALE,
                            )
                            if dcg_i % 2 == 1:
                                pending_f.append((
                                    (f_ps[ib][:], ones2[:, 0:2, 0:1],
                                     sq[:, 0:2, 0:IB]),
                                    dict(start=(dcg_i == 1),
                                         stop=(dcg_i == KC - 1),
                                         perf_mode=DR),
                                ))
                    if sub == 0:
                        # prefetch first G slice mid-conv
                        j0 = JSLICES[0]
                        cb0_t = cbp0.tile([128, KCC, j0], F8)
                        nc.sync.dma_start(
                            cb0_t[:],
                            cb_d[:, 0:j0].rearrange("(cc p) j -> p cc j",
                                                    p=128),
                        )
                        # f for ib 0,1 is complete: flush its matmuls and
                        # drain to f_row now so the conv->G transition is
                        # just ib 2,3
                        for args, kw in pending_f:
                            nc.tensor.matmul(*args, **kw)
                        pending_f = []
                        for ib in (0, 1):
                            nc.vector.tensor_copy(
                                f_row[:, ib * IB:(ib + 1) * IB], f_ps[ib][:]
                            )
                for args, kw in pending_f:
                    nc.tensor.matmul(*args, **kw)
                pending_f = []
                for ib in (2, 3):
                    nc.vector.tensor_copy(
                        f_row[:, ib * IB:(ib + 1) * IB], f_ps[ib][:]
                    )

            # ------------- G phase: Y = 64(2 phi.C - c), streamed top-8 -----
            with (
                tc.tile_pool(name="cbp", bufs=2) as cbp,
                tc.tile_pool(name="cbcp", bufs=2) as cbcp,
                tc.tile_pool(name="ysb", bufs=4) as ysb,
                tc.tile_pool(name="tails", bufs=2) as tails,
                tc.tile_pool(name="yps", bufs=7, space="PSUM") as yps,
                tc.tile_pool(name="ccps", bufs=1, space="PSUM") as ccps,
            ):
                # f relayout [1,1568] -> [128,13]: emitted AFTER the first
                # G i-tile's matmuls so the PE doesn't idle at the conv->G
                # boundary waiting for the f_row copies (the transposes sit
                # behind G matmuls in the PE queue; f is only needed by the
                # tail, much later)
                def emit_frelayout():
                    ft = ccps.tile([128, 512], F32, name="cps")
                    for it in range(NIT):
                        w = 128 if it < 12 else LAST_W
                        nc.tensor.transpose(
                            ft[0:w, it:it + 1],
                            f_row[:, it * 128:it * 128 + w],
                            oner[0:1, 0:1].bitcast(F32),
                        )
                    # f_col holds 64*f so the tail subtract needs no rescale
                    nc.scalar.activation(f_col[:], ft[:, 0:NIT], AF.Copy,
                                         scale=WSCALE)

                # tail: d=sqrt(f-Y) then w0=1/(1+e^-g1+e^-g2), score=w0*d0.
                # exp(-g) ~ 1-g+g^2/2 on DVE: the top-3 gaps are < ~0.04 so
                # the cubic error is < 2e-5, and it avoids a serial ~1.3us
                # ACT exp-table load right on the kernel's critical tail.
                def emit_tail(i0, i1):
                    n = i1 - i0
                    tsl = slice(i0, i1)
                    t64 = tails.tile([128, NIT, 3], F32, tag="t64")
                    nc.vector.tensor_tensor(
                        t64[:, tsl, :],
                        f_col[:, tsl, None].broadcast_to([128, n, 3]),
                        topA[:, tsl, 0:3], ALU.subtract,
                    )
                    d3a = tails.tile([128, NIT, 3], F32, tag="d3a")
                    nc.scalar.activation(d3a[:, tsl, :], t64[:, tsl, :],
                                         AF.Sqrt, scale=1.0 / WSCALE)
                    dd = tails.tile([128, NIT, 3], F32, tag="dd")
                    nc.vector.tensor_tensor(
                        dd[:, tsl, :], d3a[:, tsl, :],
                        d3a[:, tsl, 0:1].broadcast_to([128, n, 3]),
                        ALU.subtract,
                    )
                    qq = tails.tile([128, NIT, 3], F32, tag="qq")
                    nc.vector.tensor_tensor(qq[:, tsl, :], dd[:, tsl, :],
                                            dd[:, tsl, :], ALU.mult)
                    pe1 = tails.tile([128, NIT, 3], F32, tag="pe1")
                    nc.vector.tensor_scalar(pe1[:, tsl, :], qq[:, tsl, :],
                                            0.5, 1.0, ALU.mult, ALU.add)
                    ee = tails.tile([128, NIT, 3], F32, tag="ee")
                    nc.vector.tensor_tensor(ee[:, tsl, :], pe1[:, tsl, :],
                                            dd[:, tsl, :], ALU.subtract)
                    ss = tails.tile([128, NIT], F32, tag="ss")
                    nc.vector.tensor_reduce(ss[:, tsl], ee[:, tsl, :],
                                            AX.X, ALU.add)
                    rr = tails.tile([128, NIT], F32, tag="rr")
                    nc.vector.reciprocal(rr[:, tsl], ss[:, tsl])
                    nc.vector.tensor_tensor(
                        score_col[:, tsl], d3a[:, tsl, 0], rr[:, tsl],
                        ALU.mult,
                    )
                    nc.sync.dma_start(score_d[:, tsl], score_col[:, tsl])
                joff = [0]
                for js in range(1, NJS):
                    joff.append(joff[-1] + JSLICES[js - 1])

                for js in range(NJS):
                    w_js = JSLICES[js]
                    jsl = slice(joff[js], joff[js] + w_js)
                    if js == 0:
                        cb_t = cb0_t
                    else:
                        cb_t = cbp.tile([128, KCC, w_js], F8, name="cb_t",
                                        tag="cb")
                        nc.sync.dma_start(
                            cb_t[:],
                            cb_d[:, jsl].rearrange("(cc p) j -> p cc j",
                                                   p=128),
                        )
                    # materialize -64c for this slice: ones2 @ (-c/2 block)
                    cps_t = ccps.tile([128, 512], F32, name="cps")
                    nc.tensor.matmul(cps_t[:, 0:w_js], ones2[:],
                                     cb_t[:, KC, :], start=True, stop=True)
                    cbc_t = cbcp.tile([128, 512], F32, name="cbc_t")
                    nc.scalar.activation(cbc_t[:, 0:w_js], cps_t[:, 0:w_js],
                                         AF.Copy)
                    for it in range(NIT):
                        i0 = it * 128
                        y = yps.tile([128, 512], F32, name="y", tag="y")
                        for pc in range(NPAIR):
                            nc.tensor.matmul(
                                y[:, 0:w_js],
                                phi[:, 2 * pc:2 * pc + 2, i0:i0 + 128],
                                cb_t[:, 2 * pc:2 * pc + 2, :],
                                start=(pc == 0),
                                stop=(pc == NPAIR - 1),
                                perf_mode=DR,
                            )
                        ys = ysb.tile([128, 512], F32, name="ys", tag="ys")
                        nc.vector.tensor_tensor(
                            ys[:, 0:w_js], y[:, 0:w_js],
                            cbc_t[:, 0:w_js], ALU.add,
                        )
                        # each slice owns an 8-slot block; no merge chain
                        nc.vector.max(runA[:, it, 8 * js:8 * js + 8],
                                      ys[:, 0:w_js])
                        if js == NJS - 1:
                            # top-8 of the 56 slice-winners for this tile
                            nc.vector.max(topA[:, it, :], runA[:, it, :])
                        if js == 0 and it == 0:
                            emit_frelayout()

                emit_tail(0, NIT)

    nc.compile()
    return nc


def _get_program():
    if "nc" not in _cache:
        _cache["nc"] = _build_program()
    return _cache["nc"]


def kernel(p, W, b, C):
    from concourse.bass_utils import run_bass_kernel_spmd

    nc = _get_program()

    F8NP = ml_dtypes.float8_e4m3

    p = np.ascontiguousarray(np.asarray(p, dtype=np.float32))
    W = np.asarray(W, dtype=np.float32)
    b = np.ascontiguousarray(np.asarray(b, dtype=np.float32))
    C = np.ascontiguousarray(np.asarray(C, dtype=np.float32))

    # fold the conv bias into the prototypes: ||(Wp+b) - C_j|| =
    # ||Wp - (C_j - b)||, so the device kernel needs no bias path
    Cs = C - b[:, None]

    # wt[dcg, pin, cc, d] = 64*W[dcg*128+d, cc*128+pin]
    wt = np.ascontiguousarray(
        (WSCALE * W).reshape(KC, 128, KC, 128).transpose(0, 3, 2, 1)
    ).astype(F8NP)
    cn = np.sum(Cs.astype(np.float64) * Cs, axis=0).astype(np.float32)
    cb = np.empty((KCC * 128, P), dtype=F8NP)
    cb[:DIM] = (2.0 * WSCALE * Cs).astype(F8NP)
    cb[DIM:] = np.broadcast_to((-cn / 2.0)[None, :], (128, P)).astype(F8NP)
    oner = np.ones((1, 128), dtype=np.float32)
    ones2 = np.ones((128, 2, 64), dtype=F8NP)

    p_flat = p.reshape(B, DIM, HW)
    in_maps = []
    for core in range(NCORES):
        bidx, half = divmod(core, 2)
        # ib-major, partition-major layout: pT[ib, p, cc, i] so each
        # i-block is one DMA of 128 fully-contiguous 5.6KB runs; 392-col
        # blocks padded to 400 for the DoubleRow 16B-stride rule
        pc_ = p_flat[bidx, :, half * HALF:(half + 1) * HALF].reshape(
            KC, 128, NIB, IB)
        pT = np.zeros((NIB, 128, KC, IBPAD), dtype=F8NP)
        pT[:, :, :, :IB] = pc_.transpose(2, 1, 0, 3).astype(F8NP)
        in_maps.append({
            "pT": pT, "wt": wt, "cb": cb,
            "oner": oner, "ones2": ones2,
        })

    _cache["last_in_maps"] = in_maps
    res = run_bass_kernel_spmd(nc, in_maps, list(range(NCORES)))
    _cache["last_result"] = res

    return assemble_output(per_core=[res.results[c]["score"] for c in range(NCORES)])


def assemble_output(per_core=None, res_concat=None):
    if per_core is None:
        sc_all = res_concat["score"]                              # [8*128, 13]
        per_core = [sc_all[c * 128:(c + 1) * 128] for c in range(NCORES)]
    out = np.empty((B, 1, H, W_), dtype=np.float32)
    for core in range(NCORES):
        bidx, half = divmod(core, 2)
        sc = per_core[core]                                       # [128, 13]
        flat = np.empty(HALF, dtype=np.float32)
        flat[:12 * 128] = sc[:, :12].T.reshape(-1)
        flat[12 * 128:] = sc[:LAST_W, 12]
        out.reshape(B, 1, HW)[bidx, 0, half * HALF:(half + 1) * HALF] = flat
    return out


# revision 58
# speedup vs baseline: 1.0224x; 1.0224x over previous
"""Trainium2 Bass kernel for nn_DSVDD (retrieval_knn).

Math (per batch b):
  phi = W @ p_b + bias            [DIM, HW]    (1x1 conv)
  sqdist[i,j] = ||phi_i||^2 + ||C_j||^2 - 2 phi_i . C_j
  top-3 smallest distances d0<=d1<=d2  ->  w0 = 1/(1+exp(d0-d1)+exp(d0-d2))
  score[i] = w0 * d0

Device strategy (8 cores, data-parallel over (batch, HW-half)):
  All heavy matmuls run in fp8e4 with perf_mode=DoubleRow (2 contraction
  rows/cycle, ~1.7x over fp32r at FD>=256).  Host prescales W by 64 and C
  by 128 so fp8 operands sit in the normal range (sigma ~1-3); the scale
  is divided back out on the ACT path (phi = psum/64) and in the final
  sqrt (d = sqrt(f - psum/64)).  Y[i,j] = 64*(2 phi_i . C_j - c_j); the
  -c_j correction is materialized once per j-slice via a ones-matmul on
  the replicated -c/2 block and applied by a DVE add.  top-3 smallest
  sqdist == top-3 largest Y (f_i = ||phi_i||^2 common per row).  DVE max8
  finds the top-8 largest Y per row; streamed merge over j-slices.  f_i
  via fp32 ones-matmuls over Square(psum/64) so f keeps full accuracy.
  Tail (sqrt, softmin) on ACT/DVE.  rel-err budget 2e-2 >> fp8 noise.
"""
import sys

sys.path.insert(0, "/opt/trn_rl_repo")

import numpy as np
import ml_dtypes

B, DIM, H, W_ = 4, 1792, 56, 56
HW = H * W_            # 3136
P = 3136               # prototypes
NCORES = 8
HALF = HW // 2         # 1568 positions per core
KC = DIM // 128        # 14 contraction chunks
NPAIR = KC // 2        # 7 DoubleRow pairs
KCC = KC + 1           # 15 chunks in cb (incl. replicated -c/2 block)
IB = 392               # conv i-block (moving cols)
IBPAD = 400            # p tile inner pad (DoubleRow needs 16B-mult stride)
NIB = HALF // IB       # 4
JSLICES = [480, 480, 480, 480, 480, 480, 256]   # G-phase j-slices (sum 3136)
NJS = len(JSLICES)
NIT = 13               # i-tiles: 12 full + 1 ragged(32)
LAST_W = HALF - 12 * 128   # 32
PHIPAD = NIT * 128     # 1664 (phi padded so the ragged i-tile is uniform)
WSCALE = 64.0          # host prescale on W (and 2C -> 128C)

_cache = {}


def _build_program():
    import concourse.tile as tile
    from concourse import bacc, mybir

    F32 = mybir.dt.float32
    F32R = mybir.dt.float32r
    BF16 = mybir.dt.bfloat16
    F8 = mybir.dt.float8e4
    AF = mybir.ActivationFunctionType
    ALU = mybir.AluOpType
    AX = mybir.AxisListType
    DR = mybir.MatmulPerfMode.DoubleRow

    nc = bacc.Bacc("TRN2", target_bir_lowering=False, debug=False)

    pT_d = nc.dram_tensor("pT", [DIM, NIB, IBPAD], F8, kind="ExternalInput")
    wt_d = nc.dram_tensor("wt", [KC, 128, KC, 128], F8, kind="ExternalInput")
    cb_d = nc.dram_tensor("cb", [KCC * 128, P], F8, kind="ExternalInput")
    oner_d = nc.dram_tensor("oner", [1, 128], F32R, kind="ExternalInput")
    ones2_d = nc.dram_tensor("ones2", [128, 2, 64], F8, kind="ExternalInput")
    score_d = nc.dram_tensor("score", [128, NIT], F32, kind="ExternalOutput")

    with tile.TileContext(nc) as tc:
        with (
            tc.tile_pool(name="persist", bufs=1) as persist,
            tc.tile_pool(name="cbp0", bufs=1) as cbp0,
        ):
            phi = persist.tile([128, KC, PHIPAD], F8)
            oner = persist.tile([1, 128], F32R)
            ones2 = persist.tile([128, 2, 64], F8)
            f_row = persist.tile([1, HALF], F32)
            f_col = persist.tile([128, NIT], F32)
            runA = persist.tile([128, NIT, 8 * NJS], F32)
            topA = persist.tile([128, NIT, 8], F32)
            score_col = persist.tile([128, NIT], F32)

            # ------------- conv phase: phi = W @ p + b, f = ||phi||^2 -------
            with (
                tc.tile_pool(name="pp", bufs=6) as pp,
                tc.tile_pool(name="wtp", bufs=3) as wtp,
                tc.tile_pool(name="sqp", bufs=4) as sqp,
                tc.tile_pool(name="cps", bufs=4, space="PSUM") as cps,
                tc.tile_pool(name="fps", bufs=1, space="PSUM") as fps,
            ):
                f_ps = [fps.tile([1, IB], F32, name=f"fp{ib}", tag=f"f{ib}")
                        for ib in range(NIB)]

                def load_wt(dcg):
                    t = wtp.tile([128, KC, 128], F8, name="wt_t")
                    nc.sync.dma_start(t[:], wt_d[dcg])
                    return t

                # one p tile for all 4 i-blocks; host pre-pads rows to
                # [DIM, 4, 400] so each DMA packet is an 800B contiguous run
                p_t = pp.tile([128, KC, NIB, IBPAD], F8, name="pt", bufs=1)

                def load_pquarter(cc0, cc1, ib0, ib1):
                    nc.sync.dma_start(
                        p_t[:, cc0:cc1, ib0:ib1, :],
                        pT_d[cc0 * 128:cc1 * 128, ib0:ib1, :].rearrange(
                            "(cc p) ib i -> p cc ib i", p=128),
                    )

                # startup-critical loads first: wt chunk 0, then p halves
                dcg_seq = list(range(KC)) + list(reversed(range(KC)))  # snake
                wt_tiles = {0: load_wt(dcg_seq[0])}
                wt_issued = 1

                def wt_prefetch(upto):
                    nonlocal wt_issued
                    while wt_issued < min(upto, 2 * KC):
                        if dcg_seq[wt_issued] == dcg_seq[wt_issued - 1]:
                            # snake turn: same chunk again, reuse the tile
                            wt_tiles[wt_issued] = wt_tiles[wt_issued - 1]
                        else:
                            wt_tiles[wt_issued] = load_wt(dcg_seq[wt_issued])
                        wt_issued += 1

                # startup DMA order: the first conv group needs only
                # p(cc0-13, ib0) + wt0, so load ib0 alone first; each
                # dma_start costs ~700ns issue on Sync
                load_pquarter(0, 8, 0, 1)
                load_pquarter(8, KC, 0, 1)
                # (wt0 already issued first above)
                load_pquarter(0, 8, 1, 2)
                load_pquarter(8, KC, 1, 2)
                wt_prefetch(2)
                load_pquarter(0, 8, 2, NIB)
                load_pquarter(8, KC, 2, NIB)
                wt_prefetch(3)
                nc.sync.dma_start(oner[:], oner_d[:])
                nc.sync.dma_start(ones2[:], ones2_d[:])

                # PE warmup: dummy matmuls keep HAM's activity monitor hot
                # while the first real DMAs land, so conv starts at full clock.
                warm = pp.tile([128, 512], F32R, name="warm", tag="warm", bufs=1)
                nc.vector.memset(warm[:].bitcast(F32), 1.0)
                wps = cps.tile([128, 512], F32, name="wps", tag="acc")
                for _ in range(10):
                    nc.tensor.matmul(wps[:], warm[:, 0:128], warm[:],
                                     start=True, stop=True)
                # zero the phi pad columns so the uniform last i-tile reads 0s
                nc.vector.memset(phi[:, :, HALF:PHIPAD].bitcast(F32), 0.0)

                cb0_t = None
                pending_f = []
                _sq_cur = {}
                for sub in range(2):
                    for dcg_i in range(KC):
                        pos = sub * KC + dcg_i
                        dcg = dcg_seq[pos]
                        wt_t = wt_tiles.pop(pos)
                        wt_prefetch(pos + 3)
                        for k, ib in enumerate((2 * sub, 2 * sub + 1)):
                            if k == 1 and pending_f:
                                # deferred f matmuls: deps long satisfied
                                for args, kw in pending_f:
                                    nc.tensor.matmul(*args, **kw)
                                pending_f = []
                            acc = cps.tile([128, IB], F32)
                            for pc in range(NPAIR):
                                nc.tensor.matmul(
                                    acc[:],
                                    wt_t[:, 2 * pc:2 * pc + 2, :],
                                    p_t[:, 2 * pc:2 * pc + 2, ib, 0:IB],
                                    start=(pc == 0),
                                    stop=(pc == NPAIR - 1),
                                    perf_mode=DR,
                                )
                            isl = slice(ib * IB, (ib + 1) * IB)
                            # phi = psum/64 (bias folded into C on host)
                            nc.scalar.activation(
                                phi[:, dcg, isl], acc[:], AF.Identity,
                                scale=1.0 / WSCALE,
                            )
                            # phi2 = (psum/64)^2 in fp8 (values 0..~30),
                            # paired along dcg for a DoubleRow f-matmul
                            if dcg_i % 2 == 0:
                                sq = sqp.tile([128, 2, IBPAD], F8)
                                _sq_cur[ib] = sq
                            else:
                                sq = _sq_cur[ib]
                            nc.scalar.activation(
                                sq[:, dcg_i % 2, 0:IB], acc[:], AF.Square,
                                scale=1.0 / WSCALE,
                            )
                            if dcg_i % 2 == 1:
                                pending_f.append((
                                    (f_ps[ib][:], ones2[:, 0:2, 0:1],
                                     sq[:, 0:2, 0:IB]),
                                    dict(start=(dcg_i == 1),
                                         stop=(dcg_i == KC - 1),
                                         perf_mode=DR),
                                ))
                    if sub == 0:
                        # prefetch first G slice mid-conv
                        j0 = JSLICES[0]
                        cb0_t = cbp0.tile([128, KCC, j0], F8)
                        nc.sync.dma_start(
                            cb0_t[:],
                            cb_d[:, 0:j0].rearrange("(cc p) j -> p cc j",
                                                    p=128),
                        )
                        # f for ib 0,1 is complete: flush its matmuls and
                        # drain to f_row now so the conv->G transition is
                        # just ib 2,3
                        for args, kw in pending_f:
                            nc.tensor.matmul(*args, **kw)
                        pending_f = []
                        for ib in (0, 1):
                            nc.vector.tensor_copy(
                                f_row[:, ib * IB:(ib + 1) * IB], f_ps[ib][:]
                            )
                for args, kw in pending_f:
                    nc.tensor.matmul(*args, **kw)
                pending_f = []
                for ib in (2, 3):
                    nc.vector.tensor_copy(
                        f_row[:, ib * IB:(ib + 1) * IB], f_ps[ib][:]
                    )

            # ------------- f relayout: [1, 1568] -> [128, 13] ---------------
            with tc.tile_pool(name="ftp", bufs=2, space="PSUM") as ftp:
                ft = ftp.tile([128, NIT], F32)
                for it in range(NIT):
                    w = 128 if it < 12 else LAST_W
                    nc.tensor.transpose(
                        ft[0:w, it:it + 1],
                        f_row[:, it * 128:it * 128 + w],
                        oner[0:1, 0:1].bitcast(F32),
                    )
                # f_col holds 64*f so the tail subtract needs no rescale
                nc.scalar.activation(f_col[:], ft[:], AF.Copy,
                                     scale=WSCALE)

            # ------------- G phase: Y = 64(2 phi.C - c), streamed top-8 -----
            with (
                tc.tile_pool(name="cbp", bufs=2) as cbp,
                tc.tile_pool(name="cbcp", bufs=2) as cbcp,
                tc.tile_pool(name="ysb", bufs=4) as ysb,
                tc.tile_pool(name="tails", bufs=2) as tails,
                tc.tile_pool(name="yps", bufs=7, space="PSUM") as yps,
                tc.tile_pool(name="ccps", bufs=1, space="PSUM") as ccps,
            ):
                # tail: d=sqrt(f-Y) then w0=1/(1+e^-g1+e^-g2), score=w0*d0.
                # exp(-g) ~ 1-g+g^2/2 on DVE: the top-3 gaps are < ~0.04 so
                # the cubic error is < 2e-5, and it avoids a serial ~1.3us
                # ACT exp-table load right on the kernel's critical tail.
                def emit_tail(i0, i1):
                    n = i1 - i0
                    tsl = slice(i0, i1)
                    t64 = tails.tile([128, NIT, 3], F32, tag="t64")
                    nc.vector.tensor_tensor(
                        t64[:, tsl, :],
                        f_col[:, tsl, None].broadcast_to([128, n, 3]),
                        topA[:, tsl, 0:3], ALU.subtract,
                    )
                    d3a = tails.tile([128, NIT, 3], F32, tag="d3a")
                    nc.scalar.activation(d3a[:, tsl, :], t64[:, tsl, :],
                                         AF.Sqrt, scale=1.0 / WSCALE)
                    dd = tails.tile([128, NIT, 3], F32, tag="dd")
                    nc.vector.tensor_tensor(
                        dd[:, tsl, :], d3a[:, tsl, :],
                        d3a[:, tsl, 0:1].broadcast_to([128, n, 3]),
                        ALU.subtract,
                    )
                    qq = tails.tile([128, NIT, 3], F32, tag="qq")
                    nc.vector.tensor_tensor(qq[:, tsl, :], dd[:, tsl, :],
                                            dd[:, tsl, :], ALU.mult)
                    pe1 = tails.tile([128, NIT, 3], F32, tag="pe1")
                    nc.vector.tensor_scalar(pe1[:, tsl, :], qq[:, tsl, :],
                                            0.5, 1.0, ALU.mult, ALU.add)
                    ee = tails.tile([128, NIT, 3], F32, tag="ee")
                    nc.vector.tensor_tensor(ee[:, tsl, :], pe1[:, tsl, :],
                                            dd[:, tsl, :], ALU.subtract)
                    ss = tails.tile([128, NIT], F32, tag="ss")
                    nc.vector.tensor_reduce(ss[:, tsl], ee[:, tsl, :],
                                            AX.X, ALU.add)
                    rr = tails.tile([128, NIT], F32, tag="rr")
                    nc.vector.reciprocal(rr[:, tsl], ss[:, tsl])
                    nc.vector.tensor_tensor(
                        score_col[:, tsl], d3a[:, tsl, 0], rr[:, tsl],
                        ALU.mult,
                    )
                    nc.sync.dma_start(score_d[:, tsl], score_col[:, tsl])
                joff = [0]
                for js in range(1, NJS):
                    joff.append(joff[-1] + JSLICES[js - 1])

                for js in range(NJS):
                    w_js = JSLICES[js]
                    jsl = slice(joff[js], joff[js] + w_js)
                    if js == 0:
                        cb_t = cb0_t
                    else:
                        cb_t = cbp.tile([128, KCC, w_js], F8, name="cb_t",
                                        tag="cb")
                        nc.sync.dma_start(
                            cb_t[:],
                            cb_d[:, jsl].rearrange("(cc p) j -> p cc j",
                                                   p=128),
                        )
                    # materialize -64c for this slice: ones2 @ (-c/2 block)
                    cps_t = ccps.tile([128, 512], F32, name="cps")
                    nc.tensor.matmul(cps_t[:, 0:w_js], ones2[:],
                                     cb_t[:, KC, :], start=True, stop=True)
                    cbc_t = cbcp.tile([128, 512], F32, name="cbc_t")
                    nc.scalar.activation(cbc_t[:, 0:w_js], cps_t[:, 0:w_js],
                                         AF.Copy)
                    for it in range(NIT):
                        i0 = it * 128
                        y = yps.tile([128, 512], F32, name="y", tag="y")
                        for pc in range(NPAIR):
                            nc.tensor.matmul(
                                y[:, 0:w_js],
                                phi[:, 2 * pc:2 * pc + 2, i0:i0 + 128],
                                cb_t[:, 2 * pc:2 * pc + 2, :],
                                start=(pc == 0),
                                stop=(pc == NPAIR - 1),
                                perf_mode=DR,
                            )
                        ys = ysb.tile([128, 512], F32, name="ys", tag="ys")
                        nc.vector.tensor_tensor(
                            ys[:, 0:w_js], y[:, 0:w_js],
                            cbc_t[:, 0:w_js], ALU.add,
                        )
                        # each slice owns an 8-slot block; no merge chain
                        nc.vector.max(runA[:, it, 8 * js:8 * js + 8],
                                      ys[:, 0:w_js])
                        if js == NJS - 1:
                            # top-8 of the 56 slice-winners for this tile
                            nc.vector.max(topA[:, it, :], runA[:, it, :])

                emit_tail(0, NIT)

    nc.compile()
    return nc


def _get_program():
    if "nc" not in _cache:
        _cache["nc"] = _build_program()
    return _cache["nc"]


def kernel(p, W, b, C):
    from concourse.bass_utils import run_bass_kernel_spmd

    nc = _get_program()

    F8NP = ml_dtypes.float8_e4m3

    p = np.ascontiguousarray(np.asarray(p, dtype=np.float32))
    W = np.asarray(W, dtype=np.float32)
    b = np.ascontiguousarray(np.asarray(b, dtype=np.float32))
    C = np.ascontiguousarray(np.asarray(C, dtype=np.float32))

    # fold the conv bias into the prototypes: ||(Wp+b) - C_j|| =
    # ||Wp - (C_j - b)||, so the device kernel needs no bias path
    Cs = C - b[:, None]

    # wt[dcg, pin, cc, d] = 64*W[dcg*128+d, cc*128+pin]
    wt = np.ascontiguousarray(
        (WSCALE * W).reshape(KC, 128, KC, 128).transpose(0, 3, 2, 1)
    ).astype(F8NP)
    cn = np.sum(Cs.astype(np.float64) * Cs, axis=0).astype(np.float32)
    cb = np.empty((KCC * 128, P), dtype=F8NP)
    cb[:DIM] = (2.0 * WSCALE * Cs).astype(F8NP)
    cb[DIM:] = np.broadcast_to((-cn / 2.0)[None, :], (128, P)).astype(F8NP)
    oner = np.ones((1, 128), dtype=np.float32)
    ones2 = np.ones((128, 2, 64), dtype=F8NP)

    p_flat = p.reshape(B, DIM, HW)
    in_maps = []
    for core in range(NCORES):
        bidx, half = divmod(core, 2)
        # pad each 392-col i-block to 400 so DMA runs are 800B-contiguous
        # and matmul slice bases stay 16B-aligned
        pT = np.zeros((DIM, NIB, IBPAD), dtype=F8NP)
        pT[:, :, :IB] = p_flat[
            bidx, :, half * HALF:(half + 1) * HALF].reshape(
            DIM, NIB, IB).astype(F8NP)
        in_maps.append({
            "pT": pT, "wt": wt, "cb": cb,
            "oner": oner, "ones2": ones2,
        })

    _cache["last_in_maps"] = in_maps
    res = run_bass_kernel_spmd(nc, in_maps, list(range(NCORES)))
    _cache["last_result"] = res

    return assemble_output(per_core=[res.results[c]["score"] for c in range(NCORES)])


def assemble_output(per_core=None, res_concat=None):
    if per_core is None:
        sc_all = res_concat["score"]                              # [8*128, 13]
        per_core = [sc_all[c * 128:(c + 1) * 128] for c in range(NCORES)]
    out = np.empty((B, 1, H, W_), dtype=np.float32)
    for core in range(NCORES):
        bidx, half = divmod(core, 2)
        sc = per_core[core]                                       # [128, 13]
        flat = np.empty(HALF, dtype=np.float32)
        flat[:12 * 128] = sc[:, :12].T.reshape(-1)
        flat[12 * 128:] = sc[:LAST_W, 12]
        out.reshape(B, 1, HW)[bidx, 0, half * HALF:(half + 1) * HALF] = flat
    return out
